# revision 1
# baseline (speedup 1.0000x reference)
"""Trainium2 Bass kernel for nn_DiffModule_40827959116531 (sparse_attention).

Reference computation (per batch element b):
    sv  = src @ W1 + b1                      # (L, O)
    tk  = trg @ W2 + b2                      # (N, O)
    tv  = trg @ W1 + b1                      # (N, O)
    score = sv @ tk.T / sqrt(O)              # (L, N)
    prob  = softmax(score, axis=-1)
    ctx   = prob @ tv                        # (L, O)
    h   = concat([sv, sv - ctx], -1)         # (L, 2O)
    h2  = relu(h @ W3a + b3a)                # (L, O)
    out = h2 @ W3b + b3b                     # (L, O)

Sharding: data-parallel over B=32 across 8 cores (4 batch elements per
core); weights replicated. Host-side marshalling casts activations and
weights to bf16 and pre-transposes src/trg to [D, L] so the contraction
dim lands on SBUF partitions with plain contiguous DMAs.

Per-core dataflow (bf16 operands, fp32 PSUM accumulation):
  - sv_T [O-part, L]   = matmul(lhsT=W1, rhs=srcT)  (+ b1 per-partition)
  - tk_T [O-part, N]   = matmul(lhsT=W2, rhs=trgT)  (+ b2 per-partition)
  - tv   [N-part, O]   = matmul(lhsT=trgT, rhs=W1)  (+ b1 via K=1 ones-row matmul)
  - score_T [N-part, L] = matmul(lhsT=tk_T, rhs=sv_T); e = exp(score/32)
    (softmax max-subtraction skipped: scores have std ~0.33, exp is safe)
  - denom over the partition dim via ones-column matmul; r = 1/denom is
    broadcast across partitions with a K=1 fp32 matmul.
  - ctx_T [O-part, L] = matmul(lhsT=tv, rhs=e); hl = sv_T - ctx_T * r
  - h2_T [O-part, L]  = relu(matmul(lhsT=W3a, rhs=[sv_T; hl]) + b3a)
  - out  [L-part, O]  = matmul(lhsT=h2_T, rhs=W3b) (+ b3b via K=1 matmul)
    -> written to DRAM in natural row-major layout (fp32).
"""

import math
from contextlib import ExitStack

import ml_dtypes
import numpy as np

import concourse.bass as bass
import concourse.mybir as mybir
import concourse.tile as tile
from concourse import bacc
from concourse.bass_utils import run_bass_kernel_spmd

P = 128
B_FULL = 32
N_CORES = 8
BS = B_FULL // N_CORES  # 4 batch elements per core
L = 1024
N = 1024
D = 1024
O = 1024

F32 = mybir.dt.float32
BF16 = mybir.dt.bfloat16
AF = mybir.ActivationFunctionType
NP_BF16 = ml_dtypes.bfloat16

LC = 512          # l-chunk size for phase B
N_LC = L // LC    # 2 chunks
KD = D // P       # 8 contraction tiles over D
KO = O // P       # 8 tiles over O
KN = N // P       # 8 tiles over N
K2O = 2 * O // P  # 16 tiles over 2O

INV_SQRT_O = 1.0 / math.sqrt(O)


def _load_weight(nc, dst, w_dram, ktiles):
    """DMA a (ktiles*128, 1024) bf16 weight into [128, ktiles, 1024]."""
    for k in range(ktiles):
        nc.sync.dma_start(dst[:, k, :], w_dram.ap()[k * P:(k + 1) * P, :])


def _load_st(nc, dest, dram, b):
    """DMA a pre-transposed (D, rows) bf16 activation into [128, KD, rows]."""
    for k in range(KD):
        nc.sync.dma_start(dest[:, k, :], dram.ap()[b, k * P:(k + 1) * P, :])


def _build(nc, tc):
    src_t = nc.dram_tensor("srcT", [BS, D, L], BF16, kind="ExternalInput")
    trg_t = nc.dram_tensor("trgT", [BS, D, N], BF16, kind="ExternalInput")
    w1 = nc.dram_tensor("W1bf", [D, O], BF16, kind="ExternalInput")
    w2 = nc.dram_tensor("W2bf", [D, O], BF16, kind="ExternalInput")
    w3a = nc.dram_tensor("W3abf", [2 * O, O], BF16, kind="ExternalInput")
    w3b = nc.dram_tensor("W3bbf", [O, O], BF16, kind="ExternalInput")
    b1 = nc.dram_tensor("b1", [O], F32, kind="ExternalInput")
    b2 = nc.dram_tensor("b2", [O], F32, kind="ExternalInput")
    b3a = nc.dram_tensor("b3a", [O], F32, kind="ExternalInput")
    b1bf = nc.dram_tensor("b1bf", [O], BF16, kind="ExternalInput")
    b3b_f = nc.dram_tensor("b3bf32", [O], F32, kind="ExternalInput")
    out = nc.dram_tensor("out", [BS, L, O], F32, kind="ExternalOutput")

    ctx = ExitStack()
    singles = ctx.enter_context(tc.tile_pool(name="singles", bufs=1))
    stp = ctx.enter_context(tc.tile_pool(name="stp", bufs=2))
    actp = ctx.enter_context(tc.tile_pool(name="actp", bufs=1))
    ehh = ctx.enter_context(tc.tile_pool(name="ehh", bufs=3))
    smallp = ctx.enter_context(tc.tile_pool(name="smallp", bufs=2))
    outp = ctx.enter_context(tc.tile_pool(name="outp", bufs=3))
    psum = ctx.enter_context(tc.tile_pool(name="psum", bufs=6, space="PSUM"))
    auxps = ctx.enter_context(tc.tile_pool(name="auxps", bufs=1, space="PSUM"))

    # ---- constants ----
    w1bf = singles.tile([P, KD, O], BF16)
    w2bf = singles.tile([P, KD, O], BF16)
    w3abf = singles.tile([P, K2O, O], BF16)
    w3bbf = singles.tile([P, KO, O], BF16)
    b1col = singles.tile([P, KO], F32)
    b2col = singles.tile([P, KO], F32)
    b3acol = singles.tile([P, KO], F32)
    b1full = singles.tile([P, O], BF16)    # b1 replicated on all partitions
    b3bfull = singles.tile([P, O], F32)    # b3b replicated on all partitions
    ones_col = singles.tile([P, 1], BF16)
    ones_row_f = singles.tile([1, P], F32)

    # small constants first (they gate the ACT/DVE psum drains)
    nc.sync.dma_start(b1col[:], b1.ap().rearrange("(oo op) -> op oo", op=P))
    nc.sync.dma_start(b2col[:], b2.ap().rearrange("(oo op) -> op oo", op=P))
    nc.sync.dma_start(b3acol[:], b3a.ap().rearrange("(oo op) -> op oo", op=P))
    # bias value rows replicated across all 128 partitions via stride-0 DMA
    nc.sync.dma_start(
        b1full[:], bass.AP(tensor=b1bf.ap().tensor, offset=0, ap=[[0, P], [1, O]]))
    nc.sync.dma_start(
        b3bfull[:], bass.AP(tensor=b3b_f.ap().tensor, offset=0, ap=[[0, P], [1, O]]))
    nc.vector.memset(ones_col[:], 1.0)
    nc.vector.memset(ones_row_f[:], 1.0)
    # W1 + batch-0 activations next: the first matmul groups need only these
    s_t0 = stp.tile([P, KD, L], BF16, tag="st")
    t_t0 = stp.tile([P, KD, N], BF16, tag="st")
    for k in range(KD):
        nc.sync.dma_start(w1bf[:, k, :], w1.ap()[k * P:(k + 1) * P, :])
        nc.sync.dma_start(s_t0[:, k, :], src_t.ap()[0, k * P:(k + 1) * P, :])
    _load_st(nc, t_t0, trg_t, 0)
    # remaining weights
    _load_weight(nc, w2bf, w2, KD)
    _load_weight(nc, w3abf, w3a, K2O)
    _load_weight(nc, w3bbf, w3b, KO)

    for b in range(BS):
        if b == 0:
            s_t, t_t = s_t0, t_t0
        else:
            s_t = stp.tile([P, KD, L], BF16, tag="st")
            t_t = stp.tile([P, KD, N], BF16, tag="st")
            _load_st(nc, s_t, src_t, b)
            _load_st(nc, t_t, trg_t, b)

        # ---- phase A: sv_T, tk_T (ACT drain + bias), tv (DVE drain) ----
        svt = actp.tile([P, KO, L], BF16, tag="svt")
        tkt = actp.tile([P, KO, N], BF16, tag="tkt")
        tv = actp.tile([P, KN, O], BF16, tag="tv")
        for j in range(KO):
            for lc in range(N_LC):
                ps = psum.tile([P, LC], F32)
                for k in range(KD):
                    nc.tensor.matmul(
                        ps[:], w1bf[:, k, j * P:(j + 1) * P],
                        s_t[:, k, lc * LC:(lc + 1) * LC],
                        start=(k == 0), stop=(k == KD - 1))
                nc.scalar.activation(
                    svt[:, j, lc * LC:(lc + 1) * LC], ps[:], AF.Identity,
                    bias=b1col[:, j:j + 1])
        for j in range(KO):
            for nch in range(N // LC):
                ps = psum.tile([P, LC], F32)
                for k in range(KD):
                    nc.tensor.matmul(
                        ps[:], w2bf[:, k, j * P:(j + 1) * P],
                        t_t[:, k, nch * LC:(nch + 1) * LC],
                        start=(k == 0), stop=(k == KD - 1))
                nc.scalar.activation(
                    tkt[:, j, nch * LC:(nch + 1) * LC], ps[:], AF.Identity,
                    bias=b2col[:, j:j + 1])
        for i in range(KN):
            for oc in range(O // LC):
                ps = psum.tile([P, LC], F32)
                for k in range(KD):
                    nc.tensor.matmul(
                        ps[:], t_t[:, k, i * P:(i + 1) * P],
                        w1bf[:, k, oc * LC:(oc + 1) * LC],
                        start=(k == 0), stop=(k == KD - 1))
                nc.vector.tensor_add(tv[:, i, oc * LC:(oc + 1) * LC], ps[:],
                                     b1full[:, oc * LC:(oc + 1) * LC])

        # ---- phase B: per l-chunk ----
        for lc in range(N_LC):
            lsl = slice(lc * LC, (lc + 1) * LC)
            # score_T -> e = exp(score / sqrt(O)); denominator matmuls
            # (sum over partitions via ones column) interleave per i-tile
            e = ehh.tile([P, KN, LC], BF16, tag="ehh")
            d_ps = auxps.tile([1, LC], F32, tag="dps")
            for i in range(KN):
                ps = psum.tile([P, LC], F32)
                for k in range(KO):
                    nc.tensor.matmul(
                        ps[:], tkt[:, k, i * P:(i + 1) * P], svt[:, k, lsl],
                        start=(k == 0), stop=(k == KO - 1))
                nc.scalar.activation(e[:, i, :], ps[:], AF.Exp, scale=INV_SQRT_O)
                nc.tensor.matmul(d_ps[:], ones_col[:, :1], e[:, i, :],
                                 start=(i == 0), stop=(i == KN - 1))
            # ctx_T + normalize + hl = sv_T - ctx_T/denom. The reciprocal +
            # partition-broadcast (K=1 fp32 matmul) are emitted after two ctx
            # groups so the PE has work while the DVE computes 1/denom.
            hl = ehh.tile([P, KO, LC], BF16, tag="ehh")
            ctx_ps = []
            r_sb = None
            for j in range(KO):
                ps = psum.tile([P, LC], F32)
                for i in range(KN):
                    nc.tensor.matmul(
                        ps[:], tv[:, i, j * P:(j + 1) * P], e[:, i, :],
                        start=(i == 0), stop=(i == KN - 1))
                ctx_ps.append(ps)
                if j == 1:
                    r_sb = smallp.tile([1, LC], F32, tag="rsb", bufs=1)
                    nc.vector.reciprocal(r_sb[:], d_ps[:])
                    r_ps = auxps.tile([P, LC], F32, tag="rps")
                    nc.tensor.matmul(r_ps[:], ones_row_f[:1, :], r_sb[:1, :],
                                     start=True, stop=True)
                    rbc = smallp.tile([P, LC], F32, tag="rbc", bufs=1)
                    nc.vector.tensor_copy(rbc[:], r_ps[:])
                    for jj in range(2):
                        ctxn = smallp.tile([P, LC], F32, tag="ctxn", bufs=2)
                        nc.vector.tensor_mul(ctxn[:], ctx_ps[jj][:], rbc[:])
                        nc.vector.tensor_sub(hl[:, jj, :], svt[:, jj, lsl], ctxn[:])
                elif j > 1:
                    ctxn = smallp.tile([P, LC], F32, tag="ctxn", bufs=2)
                    nc.vector.tensor_mul(ctxn[:], ps[:], rbc[:])
                    nc.vector.tensor_sub(hl[:, j, :], svt[:, j, lsl], ctxn[:])
            # fc3a: h2 = relu([sv_T; hl] contracted with W3a + b3a)
            h2 = ehh.tile([P, KO, LC], BF16, tag="ehh")
            for j2 in range(KO):
                ps = psum.tile([P, LC], F32)
                for k in range(K2O):
                    rhs = svt[:, k, lsl] if k < KO else hl[:, k - KO, :]
                    nc.tensor.matmul(
                        ps[:], w3abf[:, k, j2 * P:(j2 + 1) * P], rhs,
                        start=(k == 0), stop=(k == K2O - 1))
                nc.scalar.activation(h2[:, j2, :], ps[:], AF.Relu,
                                     bias=b3acol[:, j2:j2 + 1])
            # fc3b: out natural [l-part, o] + b3b via K=1 matmul
            for lt in range(LC // P):
                for oc in range(O // LC):
                    ps = psum.tile([P, LC], F32)
                    for k in range(KO):
                        nc.tensor.matmul(
                            ps[:], h2[:, k, lt * P:(lt + 1) * P],
                            w3bbf[:, k, oc * LC:(oc + 1) * LC],
                            start=(k == 0), stop=(k == KO - 1))
                    o_sb = outp.tile([P, LC], F32, tag="osb")
                    nc.vector.tensor_add(o_sb[:], ps[:],
                                         b3bfull[:, oc * LC:(oc + 1) * LC])
                    nc.sync.dma_start(
                        out.ap()[b, lc * LC + lt * P: lc * LC + (lt + 1) * P,
                                 oc * LC:(oc + 1) * LC],
                        o_sb[:])

    ctx.close()


_NC_CACHE = None


def _get_nc():
    global _NC_CACHE
    if _NC_CACHE is None:
        nc = bacc.Bacc("TRN2", target_bir_lowering=False, debug=False,
                       num_devices=N_CORES)
        with tile.TileContext(nc) as tc:
            _build(nc, tc)
        nc.compile()
        _NC_CACHE = nc
    return _NC_CACHE


def kernel(**inputs):
    nc = _get_nc()
    src = np.asarray(inputs["src"], dtype=np.float32)
    trg = np.asarray(inputs["trg"], dtype=np.float32)
    # host-side marshalling: bf16 cast + transpose so the contraction dim
    # (D) lands on SBUF partitions with contiguous DMAs on-device.
    src_t = np.ascontiguousarray(
        src.astype(NP_BF16).transpose(0, 2, 1))   # (B, D, L)
    trg_t = np.ascontiguousarray(
        trg.astype(NP_BF16).transpose(0, 2, 1))   # (B, D, N)
    shared = {
        "W1bf": np.ascontiguousarray(np.asarray(inputs["W1"], np.float32).astype(NP_BF16)),
        "W2bf": np.ascontiguousarray(np.asarray(inputs["W2"], np.float32).astype(NP_BF16)),
        "W3abf": np.ascontiguousarray(np.asarray(inputs["W3a"], np.float32).astype(NP_BF16)),
        "W3bbf": np.ascontiguousarray(np.asarray(inputs["W3b"], np.float32).astype(NP_BF16)),
        "b1": np.ascontiguousarray(np.asarray(inputs["b1"], np.float32)),
        "b2": np.ascontiguousarray(np.asarray(inputs["b2"], np.float32)),
        "b3a": np.ascontiguousarray(np.asarray(inputs["b3a"], np.float32)),
        "b1bf": np.ascontiguousarray(np.asarray(inputs["b1"], np.float32).astype(NP_BF16)),
        "b3bf32": np.ascontiguousarray(np.asarray(inputs["b3b"], np.float32)),
    }
    in_maps = []
    for c in range(N_CORES):
        m = dict(shared)
        m["srcT"] = src_t[c * BS:(c + 1) * BS]
        m["trgT"] = trg_t[c * BS:(c + 1) * BS]
        in_maps.append(m)
    res = run_bass_kernel_spmd(nc, in_maps, core_ids=list(range(N_CORES)))
    return np.concatenate([r["out"] for r in res.results], axis=0)



# revision 6
# speedup vs baseline: 1.5989x; 1.5989x over previous
"""Trainium2 Bass kernel for nn_DiffModule_40827959116531 (sparse_attention).

Reference (per batch element):
    sv  = src @ W1 + b1;  tk = trg @ W2 + b2;  tv = trg @ W1 + b1
    score = sv @ tk.T / sqrt(O);  prob = softmax(score)
    ctx = prob @ tv;  h = [sv, sv - ctx]
    out = relu(h @ W3a + b3a) @ W3b + b3b

Algebraic restructuring (host-precomputed fused weights; exact up to the
shift-invariance of softmax which absorbs the b2 term):
    W12   = W1 @ W2.T               score = (src @ W12) @ trg.T + beta
    beta  = trg @ (W2 @ b1) + b1.b2         (per-target logit bias)
    Wfuse = W1 @ (W3aTop + W3aBot)  h @ W3a = src@Wfuse - ctx@W3aBot + bias
    Wcorr = W1 @ W3aBot             ctx@W3aBot = ((e@trg)/denom) @ Wcorr + ..
    bh2   = b1 @ W3aTop + b3a
This cuts the 8 matmul-units/batch to 6, and the 4 units feeding only the
softmax/correction path (g, score, ctxd, corr) tolerate fp8 -> run them as
fp8e4 DoubleRow (2 K-chunks per instruction). Only pre=src@Wfuse and the
final h2@W3b stay bf16. Verified vs reference in fp64/numpy: rel ~4e-3.

Scaling (fp8e4 min-normal 2^-6, TRN max +-240): W12, Wcorr pre-scaled by
4096 on host; g stored x8 (drain scale 2^-9); exp drain scale 2^-8; ctxd
stored x0.5; the 1/(0.5*4096) descale folds into the reciprocal-broadcast
constant 2^-11.

Sharding: data-parallel over B=32 across 8 cores (4 batch elems each).
"""

import math
from contextlib import ExitStack

import ml_dtypes
import numpy as np

import concourse.bass as bass
import concourse.mybir as mybir
import concourse.tile as tile
from concourse import bacc
from concourse.bass_utils import run_bass_kernel_spmd

P = 128
B_FULL = 32
N_CORES = 8
BS = B_FULL // N_CORES  # 4 batch elements per core
L = 1024
N = 1024
D = 1024
O = 1024

F32 = mybir.dt.float32
BF16 = mybir.dt.bfloat16
FP8 = mybir.dt.float8e4
AF = mybir.ActivationFunctionType
DR = mybir.MatmulPerfMode.DoubleRow
NP_BF16 = ml_dtypes.bfloat16
NP_FP8 = ml_dtypes.float8_e4m3fn

LC = 512
N_LC = L // LC            # 2 moving chunks of 512
KT = 8                    # 128-tiles along any contraction dim
KP = KT // 2              # DoubleRow pairs

WS = 4096.0               # host pre-scale on W12 / Wcorr
GS = 8.0                  # g storage scale
CS = 0.5                  # ctxd storage scale
G_DRAIN = GS / WS                     # 2^-9
E_DRAIN = 1.0 / (GS * math.sqrt(O))   # 2^-8
RBC_CONST = 1.0 / (CS * WS)           # 2^-11, folded into r broadcast


def _load_w(nc, dst, w_dram, ktiles):
    for k in range(ktiles):
        nc.sync.dma_start(dst[:, k, :], w_dram.ap()[k * P:(k + 1) * P, :])


def _load_act(nc, dest, dram, b):
    for k in range(KT):
        nc.sync.dma_start(dest[:, k, :], dram.ap()[b, k * P:(k + 1) * P, :])


def _build(nc, tc):
    src8_d = nc.dram_tensor("srcT8", [BS, D, L], FP8, kind="ExternalInput")
    srcb_d = nc.dram_tensor("srcTb", [BS, D, L], BF16, kind="ExternalInput")
    trgT_d = nc.dram_tensor("trgT8", [BS, D, N], FP8, kind="ExternalInput")
    trgN_d = nc.dram_tensor("trgN8", [BS, N, D], FP8, kind="ExternalInput")
    w12_d = nc.dram_tensor("W12s", [D, D], FP8, kind="ExternalInput")
    wfuse_d = nc.dram_tensor("Wfuse", [D, O], BF16, kind="ExternalInput")
    wcorr_d = nc.dram_tensor("Wcorrs", [D, O], FP8, kind="ExternalInput")
    w3b_d = nc.dram_tensor("W3bb", [O, O], BF16, kind="ExternalInput")
    bh2_d = nc.dram_tensor("bh2", [O], F32, kind="ExternalInput")
    b3b_d = nc.dram_tensor("b3bf", [O], F32, kind="ExternalInput")
    beta_d = nc.dram_tensor("beta", [BS, N], F32, kind="ExternalInput")
    out = nc.dram_tensor("out", [BS, L, O], F32, kind="ExternalOutput")

    ctx = ExitStack()
    singles = ctx.enter_context(tc.tile_pool(name="singles", bufs=1))
    stp8 = ctx.enter_context(tc.tile_pool(name="stp8", bufs=2))
    stp1 = ctx.enter_context(tc.tile_pool(name="stp1", bufs=1))
    actp = ctx.enter_context(tc.tile_pool(name="actp", bufs=1))
    smallp = ctx.enter_context(tc.tile_pool(name="smallp", bufs=2))
    outp = ctx.enter_context(tc.tile_pool(name="outp", bufs=3))
    psum = ctx.enter_context(tc.tile_pool(name="psum", bufs=3, space="PSUM"))
    auxps = ctx.enter_context(tc.tile_pool(name="auxps", bufs=1, space="PSUM"))

    # ---- constants ----
    w12 = singles.tile([P, KT, D], FP8)
    wfuse = singles.tile([P, KT, O], BF16)
    wcorr = singles.tile([P, KT, O], FP8)
    w3b = singles.tile([P, KT, O], BF16)
    bh2col = singles.tile([P, KT], F32)
    b3bfull = singles.tile([P, O], F32)
    ones1 = singles.tile([P, 1], FP8)
    crow = singles.tile([1, P], F32)

    nc.sync.dma_start(bh2col[:], bh2_d.ap().rearrange("(oo op) -> op oo", op=P))
    nc.sync.dma_start(
        b3bfull[:], bass.AP(tensor=b3b_d.ap().tensor, offset=0, ap=[[0, P], [1, O]]))
    nc.vector.memset(ones1[:], 1.0)
    nc.vector.memset(crow[:], RBC_CONST)

    # batch-0 activations + W12 first (phase A of batch 0 needs only these)
    s8_0 = stp8.tile([P, KT, L], FP8, tag="s8")
    t8_0 = stp8.tile([P, KT, N], FP8, tag="t8")
    for k in range(KT):
        nc.sync.dma_start(w12[:, k, :], w12_d.ap()[k * P:(k + 1) * P, :])
        nc.sync.dma_start(s8_0[:, k, :], src8_d.ap()[0, k * P:(k + 1) * P, :])
    _load_act(nc, t8_0, trgT_d, 0)
    _load_w(nc, wfuse, wfuse_d, KT)
    _load_w(nc, wcorr, wcorr_d, KT)
    _load_w(nc, w3b, w3b_d, KT)

    for b in range(BS):
        if b == 0:
            s8, t8 = s8_0, t8_0
        else:
            s8 = stp8.tile([P, KT, L], FP8, tag="s8")
            t8 = stp8.tile([P, KT, N], FP8, tag="t8")
            _load_act(nc, s8, src8_d, b)
            _load_act(nc, t8, trgT_d, b)
        sb = stp1.tile([P, KT, L], BF16, tag="sb")
        tn = stp1.tile([P, KT, N], FP8, tag="tn")
        _load_act(nc, sb, srcb_d, b)
        _load_act(nc, tn, trgN_d, b)
        betac = smallp.tile([P, KT], F32, tag="beta")
        nc.sync.dma_start(
            betac[:], beta_d.ap()[b].rearrange("(no np) -> np no", np=P))

        # ---- A: g_T[d2, l] = W12s.T @ srcT8, drain x 2^-9 -> fp8 (x GS) ----
        g8 = actp.tile([P, KT, L], FP8, tag="g8")
        for j in range(KT):
            pss = [psum.tile([P, LC], F32, name=f"ps{lcx}", tag=f"ps{lcx}")
                   for lcx in range(N_LC)]
            for kp in range(KP):
                for lc in range(N_LC):
                    nc.tensor.matmul(
                        pss[lc][:], w12[:, 2 * kp:2 * kp + 2, j * P:(j + 1) * P],
                        s8[:, 2 * kp:2 * kp + 2, lc * LC:(lc + 1) * LC],
                        start=(kp == 0), stop=(kp == KP - 1), perf_mode=DR)
            for lc in range(N_LC):
                nc.scalar.activation(
                    g8[:, j, lc * LC:(lc + 1) * LC], pss[lc][:], AF.Identity,
                    scale=G_DRAIN)

        # ---- B: score_T[n, l]; e = exp(score/32 + beta) -> fp8 ----
        e8 = actp.tile([P, KT, L], FP8, tag="e8")
        for i in range(KT):
            pss = [psum.tile([P, LC], F32, name=f"ps{lcx}", tag=f"ps{lcx}")
                   for lcx in range(N_LC)]
            for kp in range(KP):
                for lc in range(N_LC):
                    nc.tensor.matmul(
                        pss[lc][:], t8[:, 2 * kp:2 * kp + 2, i * P:(i + 1) * P],
                        g8[:, 2 * kp:2 * kp + 2, lc * LC:(lc + 1) * LC],
                        start=(kp == 0), stop=(kp == KP - 1), perf_mode=DR)
            for lc in range(N_LC):
                nc.scalar.activation(
                    e8[:, i, lc * LC:(lc + 1) * LC], pss[lc][:], AF.Exp,
                    scale=E_DRAIN, bias=betac[:, i:i + 1])
        # denominator: ones-column partition-reduce per N-tile
        rbcs = []
        for lc in range(N_LC):
            d_ps = auxps.tile([1, LC], F32, tag="dps")
            for i in range(KT):
                nc.tensor.matmul(
                    d_ps[:], ones1[:, :1],
                    e8[:, i, lc * LC:(lc + 1) * LC],
                    start=(i == 0), stop=(i == KT - 1))
            r_sb = smallp.tile([1, LC], F32, tag="rsb")
            nc.vector.reciprocal(r_sb[:], d_ps[:])
            r_ps = auxps.tile([P, LC], F32, tag="rps")
            nc.tensor.matmul(r_ps[:], crow[:1, :], r_sb[:1, :],
                             start=True, stop=True)
            rbc = smallp.tile([P, LC], F32, tag="rbc")
            nc.vector.tensor_copy(rbc[:], r_ps[:])
            rbcs.append(rbc)

        # ---- E: pre_T[o, l] = Wfuse.T @ srcTb + bh2 (bf16, independent) ----
        pre = actp.tile([P, KT, L], BF16, tag="pre")
        for j in range(KT):
            pss = [psum.tile([P, LC], F32, name=f"ps{lcx}", tag=f"ps{lcx}")
                   for lcx in range(N_LC)]
            for k in range(KT):
                for lc in range(N_LC):
                    nc.tensor.matmul(
                        pss[lc][:], wfuse[:, k, j * P:(j + 1) * P],
                        sb[:, k, lc * LC:(lc + 1) * LC],
                        start=(k == 0), stop=(k == KT - 1))
            for lc in range(N_LC):
                nc.scalar.activation(
                    pre[:, j, lc * LC:(lc + 1) * LC], pss[lc][:], AF.Identity,
                    bias=bh2col[:, j:j + 1])

        # ---- C: ctxd_T[d, l] = trgN8.T @ e8, drain x 0.5 -> fp8 ----
        cx8 = actp.tile([P, KT, L], FP8, tag="cx8")
        for j in range(KT):
            pss = [psum.tile([P, LC], F32, name=f"ps{lcx}", tag=f"ps{lcx}")
                   for lcx in range(N_LC)]
            for ip in range(KP):
                for lc in range(N_LC):
                    nc.tensor.matmul(
                        pss[lc][:], tn[:, 2 * ip:2 * ip + 2, j * P:(j + 1) * P],
                        e8[:, 2 * ip:2 * ip + 2, lc * LC:(lc + 1) * LC],
                        start=(ip == 0), stop=(ip == KP - 1), perf_mode=DR)
            for lc in range(N_LC):
                nc.scalar.activation(
                    cx8[:, j, lc * LC:(lc + 1) * LC], pss[lc][:], AF.Identity,
                    scale=CS)

        # ---- D: corr; h2 = relu(pre - corr/denom) -> bf16 ----
        h2 = actp.tile([P, KT, L], BF16, tag="h2")
        for j in range(KT):
            pss = [psum.tile([P, LC], F32, name=f"ps{lcx}", tag=f"ps{lcx}")
                   for lcx in range(N_LC)]
            for kp in range(KP):
                for lc in range(N_LC):
                    nc.tensor.matmul(
                        pss[lc][:], wcorr[:, 2 * kp:2 * kp + 2, j * P:(j + 1) * P],
                        cx8[:, 2 * kp:2 * kp + 2, lc * LC:(lc + 1) * LC],
                        start=(kp == 0), stop=(kp == KP - 1), perf_mode=DR)
            for lc in range(N_LC):
                lsl = slice(lc * LC, (lc + 1) * LC)
                tmp = smallp.tile([P, LC], F32, tag="tmp")
                nc.vector.tensor_mul(tmp[:], pss[lc][:], rbcs[lc][:])
                hsum = smallp.tile([P, LC], F32, tag="hsum")
                nc.vector.tensor_sub(hsum[:], pre[:, j, lsl], tmp[:])
                nc.scalar.activation(h2[:, j, lsl], hsum[:], AF.Relu)

        # ---- F: out[l, o] = h2.T @ W3b + b3b ----
        for lc in range(N_LC):
            for lt in range(LC // P):
                lab = lc * LC + lt * P
                pss = [psum.tile([P, LC], F32, name=f"ps{lcx}", tag=f"ps{lcx}")
                   for lcx in range(N_LC)]
                for k in range(KT):
                    for oc in range(O // LC):
                        nc.tensor.matmul(
                            pss[oc][:], h2[:, k, lab:lab + P],
                            w3b[:, k, oc * LC:(oc + 1) * LC],
                            start=(k == 0), stop=(k == KT - 1))
                for oc in range(O // LC):
                    o_sb = outp.tile([P, LC], F32, tag="osb")
                    nc.vector.tensor_add(o_sb[:], pss[oc][:],
                                         b3bfull[:, oc * LC:(oc + 1) * LC])
                    nc.sync.dma_start(
                        out.ap()[b, lab:lab + P, oc * LC:(oc + 1) * LC], o_sb[:])

    ctx.close()


_NC_CACHE = None


def _get_nc():
    global _NC_CACHE
    if _NC_CACHE is None:
        nc = bacc.Bacc("TRN2", target_bir_lowering=False, debug=False,
                       num_devices=N_CORES)
        with tile.TileContext(nc) as tc:
            _build(nc, tc)
        nc.compile()
        _NC_CACHE = nc
    return _NC_CACHE


def _q8(x, scale=1.0):
    y = np.asarray(x, np.float32) * np.float32(scale)
    np.clip(y, -240.0, 240.0, out=y)
    return y.astype(NP_FP8)


def kernel(**inputs):
    nc = _get_nc()
    src = np.asarray(inputs["src"], dtype=np.float32)
    trg = np.asarray(inputs["trg"], dtype=np.float32)
    W1 = np.asarray(inputs["W1"], np.float32)
    b1 = np.asarray(inputs["b1"], np.float32)
    W2 = np.asarray(inputs["W2"], np.float32)
    b2 = np.asarray(inputs["b2"], np.float32)
    W3a = np.asarray(inputs["W3a"], np.float32)
    b3a = np.asarray(inputs["b3a"], np.float32)
    W3b = np.asarray(inputs["W3b"], np.float32)
    b3b = np.asarray(inputs["b3b"], np.float32)

    W3aT, W3aB = W3a[:O], W3a[O:]
    W12 = W1 @ W2.T
    Wfuse = W1 @ (W3aT + W3aB)
    Wcorr = W1 @ W3aB
    bh2 = b1 @ W3aT + b3a
    beta = (trg @ (W2 @ b1) + np.dot(b1, b2)).astype(np.float32)  # (B, N)

    src_t = np.ascontiguousarray(src.transpose(0, 2, 1))   # (B, D, L)
    trg_t = np.ascontiguousarray(trg.transpose(0, 2, 1))   # (B, D, N)
    shared = {
        "W12s": np.ascontiguousarray(_q8(W12, WS)),
        "Wfuse": np.ascontiguousarray(Wfuse.astype(NP_BF16)),
        "Wcorrs": np.ascontiguousarray(_q8(Wcorr, WS)),
        "W3bb": np.ascontiguousarray(W3b.astype(NP_BF16)),
        "bh2": np.ascontiguousarray(bh2),
        "b3bf": np.ascontiguousarray(b3b),
    }
    src_t8 = _q8(src_t)
    src_tb = src_t.astype(NP_BF16)
    trg_t8 = _q8(trg_t)
    trg_n8 = _q8(trg)
    in_maps = []
    for c in range(N_CORES):
        m = dict(shared)
        s = slice(c * BS, (c + 1) * BS)
        m["srcT8"] = src_t8[s]
        m["srcTb"] = src_tb[s]
        m["trgT8"] = trg_t8[s]
        m["trgN8"] = trg_n8[s]
        m["beta"] = np.ascontiguousarray(beta[s])
        in_maps.append(m)
    res = run_bass_kernel_spmd(nc, in_maps, core_ids=list(range(N_CORES)))
    return np.concatenate([r["out"] for r in res.results], axis=0)


# revision 10
# speedup vs baseline: 2.2062x; 1.3798x over previous
"""Trainium2 Bass kernel for nn_DiffModule_40827959116531 (sparse_attention).

Reference (per batch element):
    sv  = src @ W1 + b1;  tk = trg @ W2 + b2;  tv = trg @ W1 + b1
    score = sv @ tk.T / sqrt(O);  prob = softmax(score)
    ctx = prob @ tv;  h = [sv, sv - ctx]
    out = relu(h @ W3a + b3a) @ W3b + b3b

Algebraic restructuring (host-precomputed fused weights; exact up to the
shift-invariance of softmax which absorbs the b2 term):
    W12   = W1 @ W2.T               score = (src @ W12) @ trg.T + beta
    beta  = trg @ (W2 @ b1) + b1.b2         (per-target logit bias)
    Wfuse = W1 @ (W3aTop + W3aBot)  h @ W3a = src@Wfuse - ctx@W3aBot + bias
    Wcorr = W1 @ W3aBot             ctx@W3aBot = ((e@trg)/denom) @ Wcorr + ..
    bh2   = b1 @ W3aTop + b3a
This cuts the 8 matmul-units/batch to 6, and the 4 units feeding only the
softmax/correction path (g, score, ctxd, corr) tolerate fp8 -> run them as
fp8e4 DoubleRow (2 K-chunks per instruction). Only pre=src@Wfuse and the
final h2@W3b stay bf16. Verified vs reference in numpy: rel ~4e-3.

Scaling (fp8e4 min-normal 2^-6, TRN max +-240): W12 pre-scaled 4096, Wcorr
256; g stored x8 (drain scale 2^-9); exp drain scale 2^-8; ctxd stored
x0.5. The denominator ones-matrix holds 128.0 so reciprocal(d_ps) is
exactly the corr multiplier 1/(128*denom) broadcast on all partitions.

Sharding: data-parallel over B=32 across 8 cores (4 batch elems each).
"""

import math
from contextlib import ExitStack

import ml_dtypes
import numpy as np

import concourse.bass as bass
import concourse.mybir as mybir
import concourse.tile as tile
from concourse import bacc
from concourse.bass_utils import run_bass_kernel_spmd

P = 128
B_FULL = 32
N_CORES = 8
BS = B_FULL // N_CORES  # 4 batch elements per core
L = 1024
N = 1024
D = 1024
O = 1024

F32 = mybir.dt.float32
BF16 = mybir.dt.bfloat16
FP8 = mybir.dt.float8e4
AF = mybir.ActivationFunctionType
DR = mybir.MatmulPerfMode.DoubleRow
NP_BF16 = ml_dtypes.bfloat16
NP_FP8 = ml_dtypes.float8_e4m3fn

LC = 512
N_LC = L // LC            # 2 moving chunks of 512
KT = 8                    # 128-tiles along any contraction dim
KP = KT // 2              # DoubleRow pairs

WS = 4096.0               # host pre-scale on W12
WCS = 256.0               # host pre-scale on Wcorr
GS = 8.0                  # g storage scale
CS = 0.5                  # ctxd storage scale
G_DRAIN = GS / WS                     # 2^-9
E_DRAIN = 1.0 / (GS * math.sqrt(O))   # 2^-8


def _load_w(nc, dst, w_dram, ktiles):
    for k in range(ktiles):
        nc.sync.dma_start(dst[:, k, :], w_dram.ap()[k * P:(k + 1) * P, :])


def _load_act(nc, dest, dram, b):
    for k in range(KT):
        nc.sync.dma_start(dest[:, k, :], dram.ap()[b, k * P:(k + 1) * P, :])


def _build(nc, tc):
    src8_d = nc.dram_tensor("srcT8", [BS, D, L], FP8, kind="ExternalInput")
    srcb_d = nc.dram_tensor("srcTb", [BS, D, L], BF16, kind="ExternalInput")
    trgT_d = nc.dram_tensor("trgT8", [BS, D, N], FP8, kind="ExternalInput")
    trgN_d = nc.dram_tensor("trgN8", [BS, N, D], FP8, kind="ExternalInput")
    w12_d = nc.dram_tensor("W12s", [D, D], FP8, kind="ExternalInput")
    wfuse_d = nc.dram_tensor("Wfuse", [D, O], BF16, kind="ExternalInput")
    wcorr_d = nc.dram_tensor("Wcorrs", [D, O], FP8, kind="ExternalInput")
    w3b_d = nc.dram_tensor("W3bb", [O, O], BF16, kind="ExternalInput")
    bh2_d = nc.dram_tensor("bh2", [O], F32, kind="ExternalInput")
    b3b_d = nc.dram_tensor("b3bf", [O], F32, kind="ExternalInput")
    beta_d = nc.dram_tensor("beta", [BS, N], F32, kind="ExternalInput")
    out = nc.dram_tensor("out", [BS, L, O], F32, kind="ExternalOutput")

    ctx = ExitStack()
    singles = ctx.enter_context(tc.tile_pool(name="singles", bufs=1))
    stp8 = ctx.enter_context(tc.tile_pool(name="stp8", bufs=2))
    stp1 = ctx.enter_context(tc.tile_pool(name="stp1", bufs=1))
    actp = ctx.enter_context(tc.tile_pool(name="actp", bufs=1))
    smallp = ctx.enter_context(tc.tile_pool(name="smallp", bufs=2))
    outp = ctx.enter_context(tc.tile_pool(name="outp", bufs=6))
    psum = ctx.enter_context(tc.tile_pool(name="psum", bufs=3, space="PSUM"))
    auxps = ctx.enter_context(tc.tile_pool(name="auxps", bufs=2, space="PSUM"))

    # ---- constants ----
    w12 = singles.tile([P, KT, D], FP8)
    wfuse = singles.tile([P, KT, O], BF16)
    wcorr = singles.tile([P, KT, O], FP8)
    w3b = singles.tile([P, KT, O], BF16)
    bh2col = singles.tile([P, KT], F32)
    b3bfull = singles.tile([P, O], F32)
    betafull = singles.tile([P, BS * KT], F32)
    onesbig = singles.tile([P, 2, N], FP8)

    nc.sync.dma_start(bh2col[:], bh2_d.ap().rearrange("(oo op) -> op oo", op=P))
    nc.sync.dma_start(
        betafull[:], beta_d.ap().rearrange("b (no np) -> np (b no)", np=P))
    nc.sync.dma_start(
        b3bfull[:], bass.AP(tensor=b3b_d.ap().tensor, offset=0, ap=[[0, P], [1, O]]))
    nc.vector.memset(onesbig[:], 128.0)

    def phase_a(s8, g8, lcs):
        """g_T[d2, l] = W12s.T @ srcT8, drain x 2^-9 -> fp8 (x GS)."""
        for j in range(KT):
            pss = {lcx: psum.tile([P, LC], F32, name=f"ps{lcx}", tag=f"ps{lcx}")
                   for lcx in lcs}
            for kp in range(KP):
                for lc in lcs:
                    nc.tensor.matmul(
                        pss[lc][:], w12[:, 2 * kp:2 * kp + 2, j * P:(j + 1) * P],
                        s8[:, 2 * kp:2 * kp + 2, lc * LC:(lc + 1) * LC],
                        start=(kp == 0), stop=(kp == KP - 1), perf_mode=DR)
            for lc in lcs:
                nc.scalar.activation(
                    g8[:, j, lc * LC:(lc + 1) * LC], pss[lc][:], AF.Identity,
                    scale=G_DRAIN)

    # batch-0: W12 + the first half of srcT8 go first so phase A can start
    # after ~3MB; the A(lc=0) matmuls are emitted before the remaining loads.
    s8_0 = stp8.tile([P, KT, L], FP8, tag="s8")
    t8_0 = stp8.tile([P, KT, N], FP8, tag="t8")
    for k in range(KT):
        nc.sync.dma_start(w12[:, k, :], w12_d.ap()[k * P:(k + 1) * P, :])
        nc.sync.dma_start(
            s8_0[:, k, 0:LC], src8_d.ap()[0, k * P:(k + 1) * P, 0:LC])
    g8_0 = actp.tile([P, KT, L], FP8, tag="g8")
    phase_a(s8_0, g8_0, [0])
    for k in range(KT):
        nc.sync.dma_start(
            s8_0[:, k, LC:L], src8_d.ap()[0, k * P:(k + 1) * P, LC:L])
    phase_a(s8_0, g8_0, [1])
    _load_act(nc, t8_0, trgT_d, 0)
    tn_0 = stp1.tile([P, KT, N], FP8, tag="tn", bufs=2)
    _load_act(nc, tn_0, trgN_d, 0)
    _load_w(nc, wfuse, wfuse_d, KT)
    sb_0 = stp1.tile([P, KT, L], BF16, tag="sb")
    _load_act(nc, sb_0, srcb_d, 0)
    _load_w(nc, wcorr, wcorr_d, KT)
    _load_w(nc, w3b, w3b_d, KT)

    nxt = dict(s8=s8_0, t8=t8_0, sb=sb_0, tn=tn_0)
    for b in range(BS):
        s8, t8, sb, tn = nxt["s8"], nxt["t8"], nxt["sb"], nxt["tn"]

        # ---- A ----
        if b == 0:
            g8 = g8_0   # emitted above, interleaved with the prologue DMAs
        else:
            g8 = actp.tile([P, KT, L], FP8, tag="g8")
            phase_a(s8, g8, list(range(N_LC)))

        # ---- B: score_T[n, l]; e = exp(score/32 + beta) -> fp8 ----
        e8 = actp.tile([P, KT, L], FP8, tag="e8")
        for i in range(KT):
            pss = [psum.tile([P, LC], F32, name=f"ps{lcx}", tag=f"ps{lcx}")
                   for lcx in range(N_LC)]
            for kp in range(KP):
                for lc in range(N_LC):
                    nc.tensor.matmul(
                        pss[lc][:], t8[:, 2 * kp:2 * kp + 2, i * P:(i + 1) * P],
                        g8[:, 2 * kp:2 * kp + 2, lc * LC:(lc + 1) * LC],
                        start=(kp == 0), stop=(kp == KP - 1), perf_mode=DR)
            for lc in range(N_LC):
                nc.scalar.activation(
                    e8[:, i, lc * LC:(lc + 1) * LC], pss[lc][:], AF.Exp,
                    scale=E_DRAIN, bias=betafull[:, b * KT + i:b * KT + i + 1])

        # ---- C: ctxd_T[d, l] = trgN8.T @ e8, drain x 0.5 -> fp8 ----
        cx8 = actp.tile([P, KT, L], FP8, tag="cx8")
        for j in range(KT):
            pss = [psum.tile([P, LC], F32, name=f"ps{lcx}", tag=f"ps{lcx}")
                   for lcx in range(N_LC)]
            for ip in range(KP):
                for lc in range(N_LC):
                    nc.tensor.matmul(
                        pss[lc][:], tn[:, 2 * ip:2 * ip + 2, j * P:(j + 1) * P],
                        e8[:, 2 * ip:2 * ip + 2, lc * LC:(lc + 1) * LC],
                        start=(ip == 0), stop=(ip == KP - 1), perf_mode=DR)
            for lc in range(N_LC):
                nc.scalar.activation(
                    cx8[:, j, lc * LC:(lc + 1) * LC], pss[lc][:], AF.Identity,
                    scale=CS)

        if b + 1 < BS:
            nxt = dict(
                s8=stp8.tile([P, KT, L], FP8, name="s8n", tag="s8"),
                t8=stp8.tile([P, KT, N], FP8, name="t8n", tag="t8"),
                tn=stp1.tile([P, KT, N], FP8, name="tnn", tag="tn", bufs=2),
                sb=stp1.tile([P, KT, L], BF16, name="sbn", tag="sb"))
            _load_act(nc, nxt["s8"], src8_d, b + 1)
            _load_act(nc, nxt["t8"], trgT_d, b + 1)
            _load_act(nc, nxt["tn"], trgN_d, b + 1)
            _load_act(nc, nxt["sb"], srcb_d, b + 1)

        # ---- E: pre_T[o, l] = Wfuse.T @ srcTb + bh2 (bf16, independent) ----
        pre = actp.tile([P, KT, L], BF16, tag="pre")
        for j in range(KT):
            pss = [psum.tile([P, LC], F32, name=f"ps{lcx}", tag=f"ps{lcx}")
                   for lcx in range(N_LC)]
            for k in range(KT):
                for lc in range(N_LC):
                    nc.tensor.matmul(
                        pss[lc][:], wfuse[:, k, j * P:(j + 1) * P],
                        sb[:, k, lc * LC:(lc + 1) * LC],
                        start=(k == 0), stop=(k == KT - 1))
            for lc in range(N_LC):
                nc.scalar.activation(
                    pre[:, j, lc * LC:(lc + 1) * LC], pss[lc][:], AF.Identity,
                    bias=bh2col[:, j:j + 1])

        # denominator: DR ones-matrix partition-reduce (2 N-tiles per MM);
        # every d_ps row holds 128*denom, so the reciprocal lands already
        # broadcast: rbc = 2^-7/denom (2^-7 folds the Wcorr/ctxd scales).
        rbcs = []
        for lc in range(N_LC):
            d_ps = auxps.tile([P, LC], F32, tag="dps")
            for ip in range(KP):
                nc.tensor.matmul(
                    d_ps[:], onesbig[:, :, :P],
                    e8[:, 2 * ip:2 * ip + 2, lc * LC:(lc + 1) * LC],
                    start=(ip == 0), stop=(ip == KP - 1), perf_mode=DR)
            rbc = smallp.tile([P, LC], F32, tag="rbc")
            nc.vector.reciprocal(rbc[:], d_ps[:])
            rbcs.append(rbc)

        # ---- D: corr; h2 = relu(pre - corr/denom) -> bf16 ----
        h2 = actp.tile([P, KT, L], BF16, tag="h2")
        for j in range(KT):
            pss = [psum.tile([P, LC], F32, name=f"ps{lcx}", tag=f"ps{lcx}")
                   for lcx in range(N_LC)]
            for kp in range(KP):
                for lc in range(N_LC):
                    nc.tensor.matmul(
                        pss[lc][:], wcorr[:, 2 * kp:2 * kp + 2, j * P:(j + 1) * P],
                        cx8[:, 2 * kp:2 * kp + 2, lc * LC:(lc + 1) * LC],
                        start=(kp == 0), stop=(kp == KP - 1), perf_mode=DR)
            for lc in range(N_LC):
                lsl = slice(lc * LC, (lc + 1) * LC)
                tmp = smallp.tile([P, LC], F32, tag="tmp")
                nc.vector.tensor_mul(tmp[:], pss[lc][:], rbcs[lc][:])
                hsum = smallp.tile([P, LC], F32, tag="hsum")
                nc.vector.tensor_sub(hsum[:], pre[:, j, lsl], tmp[:])
                nc.scalar.activation(h2[:, j, lsl], hsum[:], AF.Relu)

        # ---- F: out[l, o] = h2.T @ W3b + b3b ----
        for lc in range(N_LC):
            for lt in range(LC // P):
                lab = lc * LC + lt * P
                pss = [psum.tile([P, LC], F32, name=f"ps{lcx}", tag=f"ps{lcx}")
                       for lcx in range(N_LC)]
                for k in range(KT):
                    for oc in range(O // LC):
                        nc.tensor.matmul(
                            pss[oc][:], h2[:, k, lab:lab + P],
                            w3b[:, k, oc * LC:(oc + 1) * LC],
                            start=(k == 0), stop=(k == KT - 1))
                for oc in range(O // LC):
                    o_sb = outp.tile([P, LC], F32, tag="osb")
                    nc.vector.tensor_add(o_sb[:], pss[oc][:],
                                         b3bfull[:, oc * LC:(oc + 1) * LC])
                    nc.sync.dma_start(
                        out.ap()[b, lab:lab + P, oc * LC:(oc + 1) * LC], o_sb[:])

    ctx.close()


_NC_CACHE = None


def _get_nc():
    global _NC_CACHE
    if _NC_CACHE is None:
        nc = bacc.Bacc("TRN2", target_bir_lowering=False, debug=False,
                       num_devices=N_CORES)
        with tile.TileContext(nc) as tc:
            _build(nc, tc)
        nc.compile()
        _NC_CACHE = nc
    return _NC_CACHE


def _q8(x, scale=1.0):
    y = np.asarray(x, np.float32) * np.float32(scale)
    np.clip(y, -240.0, 240.0, out=y)
    return y.astype(NP_FP8)


def kernel(**inputs):
    nc = _get_nc()
    src = np.asarray(inputs["src"], dtype=np.float32)
    trg = np.asarray(inputs["trg"], dtype=np.float32)
    W1 = np.asarray(inputs["W1"], np.float32)
    b1 = np.asarray(inputs["b1"], np.float32)
    W2 = np.asarray(inputs["W2"], np.float32)
    b2 = np.asarray(inputs["b2"], np.float32)
    W3a = np.asarray(inputs["W3a"], np.float32)
    b3a = np.asarray(inputs["b3a"], np.float32)
    W3b = np.asarray(inputs["W3b"], np.float32)
    b3b = np.asarray(inputs["b3b"], np.float32)

    W3aT, W3aB = W3a[:O], W3a[O:]
    W12 = W1 @ W2.T
    Wfuse = W1 @ (W3aT + W3aB)
    Wcorr = W1 @ W3aB
    bh2 = b1 @ W3aT + b3a
    beta = (trg @ (W2 @ b1) + np.dot(b1, b2)).astype(np.float32)  # (B, N)

    src_t = np.ascontiguousarray(src.transpose(0, 2, 1))   # (B, D, L)
    trg_t = np.ascontiguousarray(trg.transpose(0, 2, 1))   # (B, D, N)
    shared = {
        "W12s": np.ascontiguousarray(_q8(W12, WS)),
        "Wfuse": np.ascontiguousarray(Wfuse.astype(NP_BF16)),
        "Wcorrs": np.ascontiguousarray(_q8(Wcorr, WCS)),
        "W3bb": np.ascontiguousarray(W3b.astype(NP_BF16)),
        "bh2": np.ascontiguousarray(bh2),
        "b3bf": np.ascontiguousarray(b3b),
    }
    src_t8 = _q8(src_t)
    src_tb = src_t.astype(NP_BF16)
    trg_t8 = _q8(trg_t)
    trg_n8 = _q8(trg)
    in_maps = []
    for c in range(N_CORES):
        m = dict(shared)
        s = slice(c * BS, (c + 1) * BS)
        m["srcT8"] = src_t8[s]
        m["srcTb"] = src_tb[s]
        m["trgT8"] = trg_t8[s]
        m["trgN8"] = trg_n8[s]
        m["beta"] = np.ascontiguousarray(beta[s])
        in_maps.append(m)
    res = run_bass_kernel_spmd(nc, in_maps, core_ids=list(range(N_CORES)))
    return np.concatenate([r["out"] for r in res.results], axis=0)


# revision 11
# speedup vs baseline: 2.2300x; 1.0108x over previous
"""Trainium2 Bass kernel for nn_DiffModule_40827959116531 (sparse_attention).

Reference (per batch element):
    sv  = src @ W1 + b1;  tk = trg @ W2 + b2;  tv = trg @ W1 + b1
    score = sv @ tk.T / sqrt(O);  prob = softmax(score)
    ctx = prob @ tv;  h = [sv, sv - ctx]
    out = relu(h @ W3a + b3a) @ W3b + b3b

Algebraic restructuring (host-precomputed fused weights; exact up to the
shift-invariance of softmax which absorbs the b2 term):
    W12   = W1 @ W2.T               score = (src @ W12) @ trg.T + beta
    beta  = trg @ (W2 @ b1) + b1.b2         (per-target logit bias)
    Wfuse = W1 @ (W3aTop + W3aBot)  h @ W3a = src@Wfuse - ctx@W3aBot + bias
    Wcorr = W1 @ W3aBot             ctx@W3aBot = ((e@trg)/denom) @ Wcorr + ..
    bh2   = b1 @ W3aTop + b3a
This cuts the 8 matmul-units/batch to 6, and the 4 units feeding only the
softmax/correction path (g, score, ctxd, corr) tolerate fp8 -> run them as
fp8e4 DoubleRow (2 K-chunks per instruction). Only pre=src@Wfuse and the
final h2@W3b stay bf16. Verified vs reference in numpy: rel ~4e-3.

Scaling (fp8e4 min-normal 2^-6, TRN max +-240): W12 pre-scaled 4096, Wcorr
256; g stored x8 (drain scale 2^-9); exp drain scale 2^-8; ctxd stored
x0.5. The denominator ones-matrix holds 128.0 so reciprocal(d_ps) is
exactly the corr multiplier 1/(128*denom) broadcast on all partitions.

Sharding: data-parallel over B=32 across 8 cores (4 batch elems each).
"""

import math
from contextlib import ExitStack

import ml_dtypes
import numpy as np

import concourse.bass as bass
import concourse.mybir as mybir
import concourse.tile as tile
from concourse import bacc
from concourse.bass_utils import run_bass_kernel_spmd

P = 128
B_FULL = 32
N_CORES = 8
BS = B_FULL // N_CORES  # 4 batch elements per core
L = 1024
N = 1024
D = 1024
O = 1024

F32 = mybir.dt.float32
BF16 = mybir.dt.bfloat16
FP8 = mybir.dt.float8e4
AF = mybir.ActivationFunctionType
DR = mybir.MatmulPerfMode.DoubleRow
NP_BF16 = ml_dtypes.bfloat16
NP_FP8 = ml_dtypes.float8_e4m3fn

LC = 512
N_LC = L // LC            # 2 moving chunks of 512
KT = 8                    # 128-tiles along any contraction dim
KP = KT // 2              # DoubleRow pairs

WS = 4096.0               # host pre-scale on W12
WCS = 256.0               # host pre-scale on Wcorr
GS = 8.0                  # g storage scale
CS = 0.5                  # ctxd storage scale
G_DRAIN = GS / WS                     # 2^-9
E_DRAIN = 1.0 / (GS * math.sqrt(O))   # 2^-8


def _load_w(nc, dst, w_dram, ktiles):
    for k in range(ktiles):
        nc.sync.dma_start(dst[:, k, :], w_dram.ap()[k * P:(k + 1) * P, :])


def _load_act(nc, dest, dram, b):
    for k in range(KT):
        nc.sync.dma_start(dest[:, k, :], dram.ap()[b, k * P:(k + 1) * P, :])


def _build(nc, tc):
    src8_d = nc.dram_tensor("srcT8", [BS, D, L], FP8, kind="ExternalInput")
    srcb_d = nc.dram_tensor("srcTb", [BS, D, L], BF16, kind="ExternalInput")
    trgT_d = nc.dram_tensor("trgT8", [BS, D, N], FP8, kind="ExternalInput")
    trgN_d = nc.dram_tensor("trgN8", [BS, N, D], FP8, kind="ExternalInput")
    w12_d = nc.dram_tensor("W12s", [KT, D, P], FP8, kind="ExternalInput")
    wfuse_d = nc.dram_tensor("Wfuse", [D, O], BF16, kind="ExternalInput")
    wcorr_d = nc.dram_tensor("Wcorrs", [D, O], FP8, kind="ExternalInput")
    w3b_d = nc.dram_tensor("W3bb", [O, O], BF16, kind="ExternalInput")
    bh2_d = nc.dram_tensor("bh2", [O], F32, kind="ExternalInput")
    b3b_d = nc.dram_tensor("b3bf", [O], F32, kind="ExternalInput")
    beta_d = nc.dram_tensor("beta", [BS, N], F32, kind="ExternalInput")
    out = nc.dram_tensor("out", [BS, L, O], F32, kind="ExternalOutput")

    ctx = ExitStack()
    singles = ctx.enter_context(tc.tile_pool(name="singles", bufs=1))
    stp8 = ctx.enter_context(tc.tile_pool(name="stp8", bufs=2))
    stp1 = ctx.enter_context(tc.tile_pool(name="stp1", bufs=1))
    actp = ctx.enter_context(tc.tile_pool(name="actp", bufs=1))
    smallp = ctx.enter_context(tc.tile_pool(name="smallp", bufs=2))
    outp = ctx.enter_context(tc.tile_pool(name="outp", bufs=6))
    psum = ctx.enter_context(tc.tile_pool(name="psum", bufs=3, space="PSUM"))
    auxps = ctx.enter_context(tc.tile_pool(name="auxps", bufs=2, space="PSUM"))

    # ---- constants ----
    w12 = singles.tile([P, KT, D], FP8)
    wfuse = singles.tile([P, KT, O], BF16)
    wcorr = singles.tile([P, KT, O], FP8)
    w3b = singles.tile([P, KT, O], BF16)
    bh2col = singles.tile([P, KT], F32)
    b3bfull = singles.tile([P, O], F32)
    betafull = singles.tile([P, BS * KT], F32)
    onesbig = singles.tile([P, 2, N], FP8)

    nc.sync.dma_start(bh2col[:], bh2_d.ap().rearrange("(oo op) -> op oo", op=P))
    nc.sync.dma_start(
        betafull[:], beta_d.ap().rearrange("b (no np) -> np (b no)", np=P))
    nc.sync.dma_start(
        b3bfull[:], bass.AP(tensor=b3b_d.ap().tensor, offset=0, ap=[[0, P], [1, O]]))
    nc.vector.memset(onesbig[:], 128.0)

    def phase_a(s8, g8, lcs):
        """g_T[d2, l] = W12s.T @ srcT8, drain x 2^-9 -> fp8 (x GS)."""
        for j in range(KT):
            pss = {lcx: psum.tile([P, LC], F32, name=f"ps{lcx}", tag=f"ps{lcx}")
                   for lcx in lcs}
            for kp in range(KP):
                for lc in lcs:
                    nc.tensor.matmul(
                        pss[lc][:], w12[:, 2 * kp:2 * kp + 2, j * P:(j + 1) * P],
                        s8[:, 2 * kp:2 * kp + 2, lc * LC:(lc + 1) * LC],
                        start=(kp == 0), stop=(kp == KP - 1), perf_mode=DR)
            for lc in lcs:
                nc.scalar.activation(
                    g8[:, j, lc * LC:(lc + 1) * LC], pss[lc][:], AF.Identity,
                    scale=G_DRAIN)

    # batch-0: W12 + the first half of srcT8 go first so phase A can start
    # after ~3MB; the A(lc=0) matmuls are emitted before the remaining loads.
    s8_0 = stp8.tile([P, KT, L], FP8, tag="s8")
    t8_0 = stp8.tile([P, KT, N], FP8, tag="t8")
    nc.sync.dma_start(
        w12[:, :, 0:P], w12_d.ap()[0].rearrange("(kk p) c -> p kk c", p=P))
    for k in range(KT):
        nc.sync.dma_start(
            s8_0[:, k, 0:LC], src8_d.ap()[0, k * P:(k + 1) * P, 0:LC])
    for j in range(1, KT):
        nc.sync.dma_start(
            w12[:, :, j * P:(j + 1) * P],
            w12_d.ap()[j].rearrange("(kk p) c -> p kk c", p=P))
    g8_0 = actp.tile([P, KT, L], FP8, tag="g8")
    phase_a(s8_0, g8_0, [0])
    for k in range(KT):
        nc.sync.dma_start(
            s8_0[:, k, LC:L], src8_d.ap()[0, k * P:(k + 1) * P, LC:L])
    phase_a(s8_0, g8_0, [1])
    _load_act(nc, t8_0, trgT_d, 0)
    tn_0 = stp1.tile([P, KT, N], FP8, tag="tn", bufs=2)
    _load_act(nc, tn_0, trgN_d, 0)
    _load_w(nc, wfuse, wfuse_d, KT)
    sb_0 = stp1.tile([P, KT, L], BF16, tag="sb")
    _load_act(nc, sb_0, srcb_d, 0)
    _load_w(nc, wcorr, wcorr_d, KT)
    _load_w(nc, w3b, w3b_d, KT)

    nxt = dict(s8=s8_0, t8=t8_0, sb=sb_0, tn=tn_0)
    for b in range(BS):
        s8, t8, sb, tn = nxt["s8"], nxt["t8"], nxt["sb"], nxt["tn"]

        # ---- A ----
        if b == 0:
            g8 = g8_0   # emitted above, interleaved with the prologue DMAs
        else:
            g8 = actp.tile([P, KT, L], FP8, tag="g8")
            phase_a(s8, g8, list(range(N_LC)))

        # ---- B: score_T[n, l]; e = exp(score/32 + beta) -> fp8 ----
        e8 = actp.tile([P, KT, L], FP8, tag="e8")
        for i in range(KT):
            pss = [psum.tile([P, LC], F32, name=f"ps{lcx}", tag=f"ps{lcx}")
                   for lcx in range(N_LC)]
            for kp in range(KP):
                for lc in range(N_LC):
                    nc.tensor.matmul(
                        pss[lc][:], t8[:, 2 * kp:2 * kp + 2, i * P:(i + 1) * P],
                        g8[:, 2 * kp:2 * kp + 2, lc * LC:(lc + 1) * LC],
                        start=(kp == 0), stop=(kp == KP - 1), perf_mode=DR)
            for lc in range(N_LC):
                nc.scalar.activation(
                    e8[:, i, lc * LC:(lc + 1) * LC], pss[lc][:], AF.Exp,
                    scale=E_DRAIN, bias=betafull[:, b * KT + i:b * KT + i + 1])

        # ---- C: ctxd_T[d, l] = trgN8.T @ e8, drain x 0.5 -> fp8 ----
        cx8 = actp.tile([P, KT, L], FP8, tag="cx8")
        for j in range(KT):
            pss = [psum.tile([P, LC], F32, name=f"ps{lcx}", tag=f"ps{lcx}")
                   for lcx in range(N_LC)]
            for ip in range(KP):
                for lc in range(N_LC):
                    nc.tensor.matmul(
                        pss[lc][:], tn[:, 2 * ip:2 * ip + 2, j * P:(j + 1) * P],
                        e8[:, 2 * ip:2 * ip + 2, lc * LC:(lc + 1) * LC],
                        start=(ip == 0), stop=(ip == KP - 1), perf_mode=DR)
            for lc in range(N_LC):
                nc.scalar.activation(
                    cx8[:, j, lc * LC:(lc + 1) * LC], pss[lc][:], AF.Identity,
                    scale=CS)

        if b + 1 < BS:
            nxt = dict(
                s8=stp8.tile([P, KT, L], FP8, name="s8n", tag="s8"),
                t8=stp8.tile([P, KT, N], FP8, name="t8n", tag="t8"),
                tn=stp1.tile([P, KT, N], FP8, name="tnn", tag="tn", bufs=2),
                sb=stp1.tile([P, KT, L], BF16, name="sbn", tag="sb"))
            _load_act(nc, nxt["s8"], src8_d, b + 1)
            _load_act(nc, nxt["t8"], trgT_d, b + 1)
            _load_act(nc, nxt["tn"], trgN_d, b + 1)
            _load_act(nc, nxt["sb"], srcb_d, b + 1)

        # ---- E: pre_T[o, l] = Wfuse.T @ srcTb + bh2 (bf16, independent) ----
        pre = actp.tile([P, KT, L], BF16, tag="pre")
        for j in range(KT):
            pss = [psum.tile([P, LC], F32, name=f"ps{lcx}", tag=f"ps{lcx}")
                   for lcx in range(N_LC)]
            for k in range(KT):
                for lc in range(N_LC):
                    nc.tensor.matmul(
                        pss[lc][:], wfuse[:, k, j * P:(j + 1) * P],
                        sb[:, k, lc * LC:(lc + 1) * LC],
                        start=(k == 0), stop=(k == KT - 1))
            for lc in range(N_LC):
                nc.scalar.activation(
                    pre[:, j, lc * LC:(lc + 1) * LC], pss[lc][:], AF.Identity,
                    bias=bh2col[:, j:j + 1])

        # denominator: DR ones-matrix partition-reduce (2 N-tiles per MM);
        # every d_ps row holds 128*denom, so the reciprocal lands already
        # broadcast: rbc = 2^-7/denom (2^-7 folds the Wcorr/ctxd scales).
        rbcs = []
        for lc in range(N_LC):
            d_ps = auxps.tile([P, LC], F32, tag="dps")
            for ip in range(KP):
                nc.tensor.matmul(
                    d_ps[:], onesbig[:, :, :P],
                    e8[:, 2 * ip:2 * ip + 2, lc * LC:(lc + 1) * LC],
                    start=(ip == 0), stop=(ip == KP - 1), perf_mode=DR)
            rbc = smallp.tile([P, LC], F32, tag="rbc")
            nc.vector.reciprocal(rbc[:], d_ps[:])
            rbcs.append(rbc)

        # ---- D: corr; h2 = relu(pre - corr/denom) -> bf16 ----
        h2 = actp.tile([P, KT, L], BF16, tag="h2")
        for j in range(KT):
            pss = [psum.tile([P, LC], F32, name=f"ps{lcx}", tag=f"ps{lcx}")
                   for lcx in range(N_LC)]
            for kp in range(KP):
                for lc in range(N_LC):
                    nc.tensor.matmul(
                        pss[lc][:], wcorr[:, 2 * kp:2 * kp + 2, j * P:(j + 1) * P],
                        cx8[:, 2 * kp:2 * kp + 2, lc * LC:(lc + 1) * LC],
                        start=(kp == 0), stop=(kp == KP - 1), perf_mode=DR)
            for lc in range(N_LC):
                lsl = slice(lc * LC, (lc + 1) * LC)
                tmp = smallp.tile([P, LC], F32, tag="tmp")
                nc.vector.tensor_mul(tmp[:], pss[lc][:], rbcs[lc][:])
                hsum = smallp.tile([P, LC], F32, tag="hsum")
                nc.vector.tensor_sub(hsum[:], pre[:, j, lsl], tmp[:])
                nc.scalar.activation(h2[:, j, lsl], hsum[:], AF.Relu)

        # ---- F: out[l, o] = h2.T @ W3b + b3b ----
        for lc in range(N_LC):
            for lt in range(LC // P):
                lab = lc * LC + lt * P
                pss = [psum.tile([P, LC], F32, name=f"ps{lcx}", tag=f"ps{lcx}")
                       for lcx in range(N_LC)]
                for k in range(KT):
                    for oc in range(O // LC):
                        nc.tensor.matmul(
                            pss[oc][:], h2[:, k, lab:lab + P],
                            w3b[:, k, oc * LC:(oc + 1) * LC],
                            start=(k == 0), stop=(k == KT - 1))
                for oc in range(O // LC):
                    o_sb = outp.tile([P, LC], F32, tag="osb")
                    nc.vector.tensor_add(o_sb[:], pss[oc][:],
                                         b3bfull[:, oc * LC:(oc + 1) * LC])
                    nc.sync.dma_start(
                        out.ap()[b, lab:lab + P, oc * LC:(oc + 1) * LC], o_sb[:])

    ctx.close()


_NC_CACHE = None


def _get_nc():
    global _NC_CACHE
    if _NC_CACHE is None:
        nc = bacc.Bacc("TRN2", target_bir_lowering=False, debug=False,
                       num_devices=N_CORES)
        with tile.TileContext(nc) as tc:
            _build(nc, tc)
        nc.compile()
        _NC_CACHE = nc
    return _NC_CACHE


def _q8(x, scale=1.0):
    y = np.asarray(x, np.float32) * np.float32(scale)
    np.clip(y, -240.0, 240.0, out=y)
    return y.astype(NP_FP8)


def kernel(**inputs):
    nc = _get_nc()
    src = np.asarray(inputs["src"], dtype=np.float32)
    trg = np.asarray(inputs["trg"], dtype=np.float32)
    W1 = np.asarray(inputs["W1"], np.float32)
    b1 = np.asarray(inputs["b1"], np.float32)
    W2 = np.asarray(inputs["W2"], np.float32)
    b2 = np.asarray(inputs["b2"], np.float32)
    W3a = np.asarray(inputs["W3a"], np.float32)
    b3a = np.asarray(inputs["b3a"], np.float32)
    W3b = np.asarray(inputs["W3b"], np.float32)
    b3b = np.asarray(inputs["b3b"], np.float32)

    W3aT, W3aB = W3a[:O], W3a[O:]
    W12 = W1 @ W2.T
    Wfuse = W1 @ (W3aT + W3aB)
    Wcorr = W1 @ W3aB
    bh2 = b1 @ W3aT + b3a
    beta = (trg @ (W2 @ b1) + np.dot(b1, b2)).astype(np.float32)  # (B, N)

    src_t = np.ascontiguousarray(src.transpose(0, 2, 1))   # (B, D, L)
    trg_t = np.ascontiguousarray(trg.transpose(0, 2, 1))   # (B, D, N)
    shared = {
        "W12s": np.ascontiguousarray(
            _q8(W12, WS).reshape(D, KT, P).transpose(1, 0, 2)),
        "Wfuse": np.ascontiguousarray(Wfuse.astype(NP_BF16)),
        "Wcorrs": np.ascontiguousarray(_q8(Wcorr, WCS)),
        "W3bb": np.ascontiguousarray(W3b.astype(NP_BF16)),
        "bh2": np.ascontiguousarray(bh2),
        "b3bf": np.ascontiguousarray(b3b),
    }
    src_t8 = _q8(src_t)
    src_tb = src_t.astype(NP_BF16)
    trg_t8 = _q8(trg_t)
    trg_n8 = _q8(trg)
    in_maps = []
    for c in range(N_CORES):
        m = dict(shared)
        s = slice(c * BS, (c + 1) * BS)
        m["srcT8"] = src_t8[s]
        m["srcTb"] = src_tb[s]
        m["trgT8"] = trg_t8[s]
        m["trgN8"] = trg_n8[s]
        m["beta"] = np.ascontiguousarray(beta[s])
        in_maps.append(m)
    res = run_bass_kernel_spmd(nc, in_maps, core_ids=list(range(N_CORES)))
    return np.concatenate([r["out"] for r in res.results], axis=0)
